# revision 1
# baseline (speedup 1.0000x reference)
"""AttentionPerLabelWordLevel Trainium2 kernel (8-core SPMD, batch-sharded).

Reference computation (per batch b):
  h = tanh(x @ W.T + b)                      # [T, H]
  logits = h @ C.T                           # [S, L, C]
  m = max_L(logits)                          # [S, 1, C]
  attn = softmax_C(logits - m)               # [S, L, C]
  out[s, c, :] = sum_l attn[s, l, c] * x[s, l, :]   # [S, C, H]

Shapes: B=32, T=2500 (S=100 sentences x L=25 words), H=512, C=50.
Sharding: data-parallel over batch, 4 batches per core.

Per-core layout strategy:
  - x is DMA'd once per 8-sentence wave into "packed" SBUF tiles
    [128, 512] holding 4 sentences at partition offsets 0/32/64/96
    (25 words + 7 pad rows each).
  - PE-transpose of packed x gives x^T tiles with a padded t-axis
    (32 slots per sentence); the whole middle of the pipeline
    (h^T, logits, e) lives on this padded t-axis.
  - Steps 1-2 run as float32r matmuls (4x faster than fp32); the
    PSUM->SBUF copy / activation producers round to f32r for free.
  - Softmax normalization is folded into a per-word scale of the
    attention weights after transposing e back to [t, c] layout.
  - Step 5 runs as fp32 matmuls packed 4x along K (row groups) and
    2x along M (col groups): 8 sentences concurrently in the array.
"""

import numpy as np

import concourse.bacc as bacc
import concourse.bass as bass
import concourse.tile as tile
from concourse import mybir
from concourse.bass_utils import run_bass_kernel_spmd
from concourse.masks import make_identity

F32 = mybir.dt.float32
F32R = mybir.dt.float32r
AX = mybir.AxisListType
AF = mybir.ActivationFunctionType

N_CORES = 8
B = 32
S = 100          # sentences per batch
L = 25           # words per sentence
C = 50           # classes
H = 512          # hidden
B_LOC = B // N_CORES          # batches per core
WAVE_S = 8                    # sentences per wave (2 packed tiles)
N_WAVES = 13                  # 12 full waves + 1 half wave (4 sentences)

_CACHE = {}
LAST_RESULT = None


def build_nc():
    nc = bacc.Bacc(trn_type="TRN2", target_bir_lowering=False, debug=False)
    x_d = nc.declare_dram_parameter("input_tensor", [B_LOC, S * L, H], F32, isOutput=False)
    w_d = nc.declare_dram_parameter("W", [H, H], F32, isOutput=False)
    b_d = nc.declare_dram_parameter("b", [H], F32, isOutput=False)
    c_d = nc.declare_dram_parameter("context_vector", [C, H], F32, isOutput=False)
    o_d = nc.declare_dram_parameter("out", [B_LOC, S, C, H], F32, isOutput=True)

    with tile.TileContext(nc) as tc:
        with tc.tile_pool(name="sb", bufs=1) as sb, \
             tc.tile_pool(name="consts", bufs=1) as consts, \
             tc.tile_pool(name="ps", bufs=1, space="PSUM") as ps:

            # ---------------- one-time setup ----------------
            ident = consts.tile([128, 128], F32)
            make_identity(nc, ident)

            b_sb = consts.tile([128, 4], F32)
            nc.sync.dma_start(out=b_sb, in_=b_d.rearrange("(k p) -> p k", p=128))

            # W^T tiles: W_T[i] is [i-part 128, o 512] (f32r)
            w_nat = []
            for o in range(4):
                wn = consts.tile([128, 512], F32, name=f"w_nat{o}")
                nc.sync.dma_start(out=wn, in_=w_d[o * 128:(o + 1) * 128, :])
                w_nat.append(wn)
            w_t = []
            for i in range(4):
                pw = ps.tile([128, 512], F32, tag="xt", bufs=2, name=f"pw{i}")
                for o in range(4):
                    nc.tensor.transpose(
                        pw[:, o * 128:(o + 1) * 128],
                        w_nat[o][:, i * 128:(i + 1) * 128],
                        ident,
                    )
                wt = consts.tile([128, 512], F32R, name=f"w_t{i}")
                nc.vector.tensor_copy(wt, pw)
                w_t.append(wt)

            # C^T tile: [o-part 128, o_chunk 4, c 50] (f32r)
            c_nat = consts.tile([C, 512], F32)
            nc.sync.dma_start(out=c_nat, in_=c_d[:, :])
            c_t = consts.tile([128, 4, C], F32R)
            for o in range(4):
                pc = ps.tile([128, C], F32, tag="et", bufs=1, name=f"pc{o}")
                nc.tensor.transpose(
                    pc, c_nat[:, o * 128:(o + 1) * 128], ident[:C, :C]
                )
                nc.vector.tensor_copy(c_t[:, o, :], pc)

            # ---------------- main loop ----------------
            for bi in range(B_LOC):
                for wv in range(N_WAVES):
                    s0 = wv * WAVE_S
                    ns = min(WAVE_S, S - s0)      # 8 or 4
                    G = ns // 4                   # packed tiles (2 or 1)
                    W_COLS = 32 * ns              # padded t-cols (256 or 128)

                    # -- load packed x tiles --
                    xp = []
                    for g in range(G):
                        t_ = sb.tile([128, 512], F32, tag="xp", bufs=4,
                                     name=f"xp{bi}_{wv}_{g}")
                        for jj in range(4):
                            s_ = s0 + 4 * g + jj
                            nc.sync.dma_start(
                                out=t_[32 * jj:32 * jj + L, :],
                                in_=x_d[bi, s_ * L:(s_ + 1) * L, :],
                            )
                        xp.append(t_)

                    # -- PE-transpose x -> x^T (padded t axis), 2 psum banks --
                    xt_sb = []
                    for half in range(2):          # i-chunks (2*half, 2*half+1)
                        pxt = ps.tile([128, 512], F32, tag="xt", bufs=2,
                                      name=f"pxt{bi}_{wv}_{half}")
                        for il in range(2):
                            i = 2 * half + il
                            for g in range(G):
                                nc.tensor.transpose(
                                    pxt[:, 256 * il + 128 * g:
                                        256 * il + 128 * (g + 1)],
                                    xp[g][:, i * 128:(i + 1) * 128],
                                    ident,
                                )
                        xs = sb.tile([128, 512], F32R, tag="xt_sb", bufs=4,
                                     name=f"xt_sb{bi}_{wv}_{half}")
                        nc.vector.tensor_copy(xs, pxt)
                        xt_sb.append(xs)

                    # -- step 1: h^T[o] = tanh(W @ x^T + b), f32r --
                    h = []
                    for o in range(4):
                        ph = ps.tile([128, W_COLS], F32, tag="ph", bufs=2,
                                     name=f"ph{bi}_{wv}_{o}")
                        for i in range(4):
                            rhs = xt_sb[i // 2][:, 256 * (i % 2):
                                                256 * (i % 2) + W_COLS]
                            nc.tensor.matmul(
                                ph,
                                w_t[i][:, o * 128:(o + 1) * 128],
                                rhs,
                                start=(i == 0), stop=(i == 3),
                            )
                        ht = sb.tile([128, 256], F32R, tag="h", bufs=8,
                                     name=f"h{bi}_{wv}_{o}")
                        nc.scalar.activation(
                            out=ht[:, :W_COLS], in_=ph,
                            func=AF.Tanh, bias=b_sb[:, o:o + 1], scale=1.0,
                        )
                        h.append(ht)

                    # -- step 2: logits[c, t] (f32r accumulate over o) --
                    pl = ps.tile([C, W_COLS], F32, tag="pl", bufs=1,
                                 name=f"pl{bi}_{wv}")
                    for o in range(4):
                        nc.tensor.matmul(
                            pl, c_t[:, o, :], h[o][:, :W_COLS],
                            start=(o == 0), stop=(o == 3),
                        )

                    # -- m = max over words (strided view skips pad cols) --
                    m = sb.tile([C, WAVE_S], F32, tag="m", bufs=2,
                                name=f"m{bi}_{wv}")
                    pl_v = bass.AP(tensor=pl.tensor, offset=pl.offset,
                                   ap=[pl.ap[0], [32, ns], [1, L]])
                    nc.vector.reduce_max(out=m[:, :ns], in_=pl_v, axis=AX.X)

                    # -- e = exp(logits - m) (strided, padded layout kept) --
                    epre = sb.tile([C, 256], F32, tag="epre", bufs=2,
                                   name=f"epre{bi}_{wv}")
                    e_sb = sb.tile([C, 256], F32, tag="e", bufs=2,
                                   name=f"e{bi}_{wv}")
                    ep_v = bass.AP(tensor=epre.tensor, offset=epre.offset,
                                   ap=[epre.ap[0], [32, ns], [1, L]])
                    e_v = bass.AP(tensor=e_sb.tensor, offset=e_sb.offset,
                                  ap=[e_sb.ap[0], [32, ns], [1, L]])
                    m_v = bass.AP(tensor=m.tensor, offset=m.offset,
                                  ap=[m.ap[0], [1, ns], [0, L]])
                    nc.vector.tensor_sub(ep_v, pl_v, m_v)
                    nc.scalar.activation(out=e_v, in_=ep_v, func=AF.Exp)

                    # -- transpose e -> packed attn tiles, normalize --
                    attn = []
                    for g in range(G):
                        pe_t = ps.tile([128, C], F32, tag="et", bufs=1,
                                       name=f"pet{bi}_{wv}_{g}")
                        nc.tensor.transpose(
                            pe_t, e_sb[:, 128 * g:128 * (g + 1)], ident[:C, :C]
                        )
                        at = sb.tile([128, C], F32, tag="attn", bufs=4,
                                     name=f"attn{bi}_{wv}_{g}")
                        nc.vector.tensor_copy(at, pe_t)
                        z = sb.tile([128, 1], F32, tag="z", bufs=4,
                                    name=f"z{bi}_{wv}_{g}")
                        nc.vector.reduce_sum(out=z, in_=at, axis=AX.X)
                        nc.vector.reciprocal(out=z, in_=z)
                        nc.vector.tensor_scalar_mul(at, at, z)
                        attn.append(at)

                    # -- step 5: out[c, o] per sentence; 4xK 2xM packed fp32 --
                    for w2 in range(2):
                        po_t = {}
                        for jj in (2 * w2, 2 * w2 + 1):
                            po = ps.tile([128, 512], F32, tag=f"po{jj % 2}",
                                         bufs=1, name=f"po{bi}_{wv}_{jj}")
                            for g in range(G):
                                nc.tensor.matmul(
                                    po[64 * g:64 * g + C, :],
                                    attn[g][32 * jj:32 * jj + L, :],
                                    xp[g][32 * jj:32 * jj + L, :],
                                    start=True, stop=True,
                                    tile_position=(32 * jj, 64 * g),
                                )
                            po_t[jj] = po
                        for jj in (2 * w2, 2 * w2 + 1):
                            osb = sb.tile([128, 512], F32, tag="osb", bufs=8,
                                          name=f"osb{bi}_{wv}_{jj}")
                            ncols = 64 * (G - 1) + C
                            if jj % 2 == 0:
                                nc.vector.tensor_copy(
                                    osb[:ncols, :], po_t[jj][:ncols, :])
                            else:
                                nc.scalar.copy(
                                    osb[:ncols, :], po_t[jj][:ncols, :])
                            for g in range(G):
                                s_ = s0 + 4 * g + jj
                                nc.sync.dma_start(
                                    out=o_d[bi, s_],
                                    in_=osb[64 * g:64 * g + C, :],
                                )
    nc.compile()
    return nc


def kernel(**inputs):
    global LAST_RESULT
    if "nc" not in _CACHE:
        _CACHE["nc"] = build_nc()
    nc = _CACHE["nc"]

    x = np.ascontiguousarray(inputs["input_tensor"], dtype=np.float32)
    w = np.ascontiguousarray(inputs["W"], dtype=np.float32)
    bb = np.ascontiguousarray(inputs["b"], dtype=np.float32)
    cv = np.ascontiguousarray(inputs["context_vector"], dtype=np.float32)

    in_maps = [
        {
            "input_tensor": x[ci * B_LOC:(ci + 1) * B_LOC],
            "W": w,
            "b": bb,
            "context_vector": cv,
        }
        for ci in range(N_CORES)
    ]
    res = run_bass_kernel_spmd(nc, in_maps, core_ids=list(range(N_CORES)))
    LAST_RESULT = res
    out = np.empty((B, S, C, H), dtype=np.float32)
    for ci in range(N_CORES):
        out[ci * B_LOC:(ci + 1) * B_LOC] = res.results[ci]["out"]
    return out
